# revision 24
# baseline (speedup 1.0000x reference)
"""Self-contained Trainium2 kernel for causal multi-head attention.

Module: x[4,2048,1024] -> QKV proj (16 heads, hd=64) -> causal softmax
(scale 1/sqrt(1024)) -> out [4,2048,1024].

Sharding: 8 cores = 4 batches x 2 head-groups (8 heads each). Each core is
fully independent (full seq per core, no collectives).

Per-core dataflow (transpose-free attention):
  - host pre-transposes x -> xT [1024,2048] and pre-packs W with the
    1/sqrt(d) scale folded into the Q columns; bf16 compute, fp32 PSUM.
  - QKV^T: qT/kT tiles [2*64, 2048] per head-pair via lhsT=W, rhs=xT;
    V in natural [seq, 64] layout via lhsT=xT, rhs=Wv (+bias via ones-matmul)
  - S^T[j,i] = kT_blk.T @ qT (K=64) per head; exp on ScalarE straight from
    PSUM; causal mask = multiply by 0/1 triangle on diag blocks only
  - O^T accum: lhsT=[V|1] (65 cols; col 64 gives softmax denominators free),
    rhs = exp(S^T); accumulate over j-tiles in PSUM
  - PE-transpose O^T -> O natural; DVE reciprocal + per-partition scale;
    DMA out [2048, 512] fp32 per core
"""

import sys
import types

import ml_dtypes
import numpy as np

# ---------------------------------------------------------------------------
# Environment shims (axon NTFF profile hook that this image's antenv lacks)
# ---------------------------------------------------------------------------
if "antenv.axon_hooks" not in sys.modules:
    try:
        import antenv

        try:
            from trn_agent_boot.trn_boot import _ntff_profile_via_ctypes

            _hook = _ntff_profile_via_ctypes("/opt/axon/libaxon_pjrt.so")
        except Exception:
            _hook = None
        _mod = types.ModuleType("antenv.axon_hooks")
        _mod.get_axon_ntff_profile_hook = lambda: _hook
        _mod.set_axon_ntff_profile_hook = lambda h: None
        sys.modules["antenv.axon_hooks"] = _mod
        antenv.axon_hooks = _mod
    except ImportError:
        pass

import concourse.bass as bass
import concourse.mybir as mybir
import concourse.tile as tile
from concourse.bass_utils import run_bass_kernel_spmd
from concourse.masks import make_identity

BF16 = ml_dtypes.bfloat16

T = 2048          # sequence length
D = 1024          # model dim
NH_CORE = 8       # heads per core
HD = 64           # head dim
NCORES = 8
NKC = D // 128    # contraction chunks (8)
NIC = T // 512    # 512-wide i chunks (4)
NJT = T // 128    # 128-wide j tiles (16)
F32 = mybir.dt.float32
BF = mybir.dt.bfloat16


# ---------------------------------------------------------------------------
# walrus workaround: split instructions with >1 semaphore wait into chained
# NoOps (this container's walrus rejects >1 sync-wait per instruction).
# ---------------------------------------------------------------------------
def _split_excess_waits(nc, max_waits=1):
    n_split = 0
    for f in nc.m.functions:
        for blk in f.blocks:
            new_insts = []
            for inst in blk.instructions:
                si = inst.sync_info
                if si is None or si.on_wait is None or len(si.on_wait) <= max_waits:
                    new_insts.append(inst)
                    continue
                waits = list(si.on_wait)
                movable = [w for w in waits if w.wait_mode == "sem-ge-imm"]
                fixed = [w for w in waits if w.wait_mode != "sem-ge-imm"]
                keep = max_waits - len(fixed)
                assert keep >= 0, f"{inst.name}: too many non-ge waits"
                kept = fixed + (movable[:keep] if keep > 0 else [])
                rest = movable[keep:] if keep > 0 else movable
                for i in range(0, len(rest), max_waits):
                    chunk = rest[i:i + max_waits]
                    n_split += 1
                    new_insts.append(mybir.InstNoOp(
                        name=f"I-waitsplit-{n_split}",
                        engine=inst.engine,
                        ins=[], outs=[],
                        sync_info=mybir.SyncInfo(on_wait=list(chunk), on_update=[]),
                        bass_nofuse=True,
                    ))
                inst.sync_info = mybir.SyncInfo(
                    on_wait=kept, on_update=list(si.on_update or []))
                new_insts.append(inst)
            blk.instructions = new_insts
    return n_split


# ---------------------------------------------------------------------------
# Device program
# ---------------------------------------------------------------------------
def _build_program():
    from contextlib import ExitStack

    nc = bass.Bass(target_bir_lowering=False, debug=False)
    xT_ext = nc.declare_dram_parameter("xT", [D, T], BF, isOutput=False)
    w_ext = nc.declare_dram_parameter("w", [D, 1536], BF, isOutput=False)
    bqk_ext = nc.declare_dram_parameter("b_qk", [128, 8], F32, isOutput=False)
    bv_ext = nc.declare_dram_parameter("b_v", [1, 512], BF, isOutput=False)
    out_ext = nc.declare_dram_parameter("out", [T, NH_CORE * HD], F32, isOutput=True)

    with tile.TileContext(nc) as tc, ExitStack() as ctx:
        const = ctx.enter_context(tc.tile_pool(name="const", bufs=1))
        # PSUM: "mm" slots are 2 banks ([128,1024] f32); fl/acc slots 1 bank
        psum_mm = ctx.enter_context(tc.tile_pool(name="psum_mm", bufs=2, space="PSUM"))
        psum_fl = ctx.enter_context(tc.tile_pool(name="psum_fl", bufs=1, space="PSUM"))
        psum_acc = ctx.enter_context(tc.tile_pool(name="psum_acc", bufs=3, space="PSUM"))
        p_pool = ctx.enter_context(tc.tile_pool(name="p_pool", bufs=6))
        ot_pool = ctx.enter_context(tc.tile_pool(name="ot_pool", bufs=3))
        o_pool = ctx.enter_context(tc.tile_pool(name="o_pool", bufs=3))
        r_pool = ctx.enter_context(tc.tile_pool(name="r_pool", bufs=2))

        # persistent SBUF tensors (split finely so Tile's per-tile dependency
        # tracking doesn't serialize phases)
        identb_sb = const.tile([128, 128], BF)
        xT_sb = [const.tile([128, T], BF, tag=f"xT{kc}", name=f"xT{kc}") for kc in range(NKC)]
        w_sb = [const.tile([128, 1536], BF, tag=f"w{kc}", name=f"w{kc}") for kc in range(NKC)]
        qt_sb = [const.tile([128, T], BF, tag=f"qt{gp}", name=f"qt{gp}") for gp in range(4)]
        kt_sb = [const.tile([128, T], BF, tag=f"kt{gp}", name=f"kt{gp}") for gp in range(4)]
        v_sb = [const.tile([128, NH_CORE * 65], BF, tag=f"v{jt}", name=f"v{jt}") for jt in range(NJT)]
        bqk_sb = const.tile([128, 8], F32)
        bv_sb = const.tile([1, 512], BF)
        ones_sb = const.tile([1, 128], BF)
        mask_sb = const.tile([128, 128], BF)

        # --- setup ---
        for kc in range(NKC):
            nc.gpsimd.dma_start(xT_sb[kc][:, :], xT_ext[kc * 128:(kc + 1) * 128, :])
            nc.gpsimd.dma_start(w_sb[kc][:, :], w_ext[kc * 128:(kc + 1) * 128, :])
        nc.gpsimd.dma_start(bqk_sb[:, :], bqk_ext[:, :])
        nc.gpsimd.dma_start(bv_sb[:, :], bv_ext[:, :])
        nc.vector.memset(ones_sb[:, :], 1.0)
        make_identity(nc, identb_sb[:, :])
        # causal 0/1 triangle (diagonal 128-col block): keep 1.0 where p <= f
        nc.gpsimd.memset(mask_sb[:, :], 1.0)
        nc.gpsimd.affine_select(
            out=mask_sb[:, :], in_=mask_sb[:, :],
            compare_op=mybir.AluOpType.is_ge, fill=0.0,
            base=0, pattern=[[1, 128]], channel_multiplier=-1,
        )
        for jt in range(NJT):
            nc.vector.memset(
                v_sb[jt][:, :].rearrange("p (h c) -> p h c", c=65)[:, :, 64:65], 1.0)

        def qk_tile_job(gp, qk, n):
            t_idx = 2 * gp + qk
            dest = qt_sb[gp] if qk == 0 else kt_sb[gp]
            ps = psum_fl.tile([128, 512], F32, tag="fl", name=f"flq{gp}_{qk}_{n}")
            for kc in range(NKC):
                nc.tensor.matmul(
                    ps[:, :],
                    lhsT=w_sb[kc][:, t_idx * 128:(t_idx + 1) * 128],
                    rhs=xT_sb[kc][:, n * 512:(n + 1) * 512],
                    start=(kc == 0), stop=(kc == NKC - 1),
                )
                yield
            nc.vector.tensor_scalar_add(
                dest[:, n * 512:(n + 1) * 512], ps[:, :],
                bqk_sb[:, t_idx:t_idx + 1],
            )
            yield

        def v_tile_job(st):
            ps = psum_fl.tile([128, 512], F32, tag="fl", name=f"flv{st}")
            nc.tensor.matmul(ps[:, :], lhsT=ones_sb[:, :], rhs=bv_sb[:, :],
                             start=True, stop=False)
            for kc in range(NKC):
                nc.tensor.matmul(
                    ps[:, :],
                    lhsT=xT_sb[kc][:, st * 128:(st + 1) * 128],
                    rhs=w_sb[kc][:, 1024:1536],
                    start=False, stop=(kc == NKC - 1),
                )
                yield
            nc.vector.tensor_copy(
                v_sb[st][:, :].rearrange("p (h c) -> p h c", c=65)[:, :, 0:64],
                ps[:, :].rearrange("p (h c) -> p h c", c=64),
            )
            yield

        est = {"pe": 0.0, "act": 0.0}

        def run_job(gen):
            for _ in gen:
                est["pe"] += 215.0

        # filler queue: [(key, generator)] pumped into phase-2 PE bubbles
        fillers = []

        def pump_one():
            while fillers:
                try:
                    next(fillers[0][1])
                    est["pe"] += 215.0
                    return True
                except StopIteration:
                    fillers.pop(0)
            return False

        def pump_balance():
            while fillers and est["pe"] < est["act"] + 2000.0:
                if not pump_one():
                    return

        def drain_through(pred):
            """Run filler jobs (FIFO) until every job matching pred is gone."""
            while any(pred(key) for key, _ in fillers):
                run_job(fillers[0][1])
                fillers.pop(0)

        pending_tail = []

        def emit_tail():
            if not pending_tail:
                return
            h, ic, ot_s = pending_tail.pop(0)
            tr_ps = psum_fl.tile([128, 264], BF, tag="fl")
            for blk in range(4):
                nc.tensor.transpose(
                    tr_ps[:, blk * 66:blk * 66 + 65],
                    ot_s[0:65, blk * 128:(blk + 1) * 128],
                    identb_sb[0:65, 0:65],
                )
            rc = r_pool.tile([128, 4], F32)
            nc.vector.reciprocal(
                rc[:, :],
                tr_ps[:, 0:264].rearrange("p (b c) -> p b c", c=66)[:, :, 64:65],
            )
            o_s = o_pool.tile([128, 256], F32)
            for blk in range(4):
                nc.vector.tensor_scalar_mul(
                    o_s[:, blk * 64:(blk + 1) * 64],
                    tr_ps[:, blk * 66:blk * 66 + 64],
                    rc[:, blk:blk + 1],
                )
            nc.gpsimd.dma_start(
                out_ext[ic * 512:(ic + 1) * 512, h * 64:(h + 1) * 64]
                .rearrange("(blk p) c -> p blk c", p=128),
                o_s[:, :].rearrange("p (blk c) -> p blk c", c=64),
            )

        def emit_unit(gp, hh, ic):
            h = 2 * gp + hh
            po = 64 * hh
            njt = 4 * ic + 4
            acc = psum_acc.tile([65, 512], F32, tag="acc")
            for jta in range(0, njt, 2):
                jtb = jta + 1
                ra = jta - 4 * ic
                rb = jtb - 4 * ic
                f0a = 128 * ra if ra >= 0 else 0
                f0b = 128 * rb if rb >= 0 else 0
                st2 = psum_mm.tile([128, 1024], F32, tag="mm")
                nc.tensor.matmul(
                    st2[:, f0a:512],
                    lhsT=kt_sb[gp][po:po + 64, jta * 128:(jta + 1) * 128],
                    rhs=qt_sb[gp][po:po + 64, ic * 512 + f0a: ic * 512 + 512],
                    start=True, stop=True,
                )
                nc.tensor.matmul(
                    st2[:, 512 + f0b:1024],
                    lhsT=kt_sb[gp][po:po + 64, jtb * 128:(jtb + 1) * 128],
                    rhs=qt_sb[gp][po:po + 64, ic * 512 + f0b: ic * 512 + 512],
                    start=True, stop=True,
                )
                pump_one()
                p_t = p_pool.tile([128, 1024], BF, tag="pt")
                if ra >= 0:
                    # diagonal pair: exp only the written regions
                    nc.scalar.activation(
                        p_t[:, f0a:512], st2[:, f0a:512],
                        mybir.ActivationFunctionType.Exp)
                    nc.scalar.activation(
                        p_t[:, 512 + f0b:1024], st2[:, 512 + f0b:1024],
                        mybir.ActivationFunctionType.Exp)
                    est["act"] += (172 + 512 - f0a) / 1.2 + (172 + 512 - f0b) / 1.2
                    nc.vector.tensor_mul(
                        p_t[:, f0a:f0a + 128], p_t[:, f0a:f0a + 128], mask_sb[:, :])
                    nc.vector.tensor_mul(
                        p_t[:, 512 + f0b:512 + f0b + 128],
                        p_t[:, 512 + f0b:512 + f0b + 128], mask_sb[:, :])
                else:
                    nc.scalar.activation(
                        p_t[:, :], st2[:, :], mybir.ActivationFunctionType.Exp)
                    est["act"] += (172 + 1024) / 1.2
                nc.tensor.matmul(
                    acc[0:65, f0a:512],
                    lhsT=v_sb[jta][:, h * 65:(h + 1) * 65],
                    rhs=p_t[:, f0a:512],
                    start=(jta == 0), stop=False,
                )
                nc.tensor.matmul(
                    acc[0:65, f0b:512],
                    lhsT=v_sb[jtb][:, h * 65:(h + 1) * 65],
                    rhs=p_t[:, 512 + f0b:1024],
                    start=False, stop=(jtb == njt - 1),
                )
                pump_one()
            ot_s = const.tile([65, 512], BF, tag=f"ot{h}_{ic}", name=f"ot{h}_{ic}")
            nc.vector.tensor_copy(ot_s[:, :], acc[:, :])
            pending_tail.append((h, ic, ot_s))

        # --- emission: qk(pair0) + V(0-3) upfront; the rest becomes filler
        # work pumped into phase-2 PE bubbles (keeps TensorE dense -> HAM warm)
        for qk in range(2):
            for n in range(NIC):
                run_job(qk_tile_job(0, qk, n))
        for st in range(4):
            run_job(v_tile_job(st))
        for st in range(4, 8):
            fillers.append((("v", st), v_tile_job(st)))
        for qk in range(2):
            for n in range(NIC):
                fillers.append((("qk", 1), qk_tile_job(1, qk, n)))
        for st in range(8, 12):
            fillers.append((("v", st), v_tile_job(st)))
        for qk in range(2):
            for n in range(NIC):
                fillers.append((("qk", 2), qk_tile_job(2, qk, n)))
        for st in range(12, 16):
            fillers.append((("v", st), v_tile_job(st)))
        for qk in range(2):
            for n in range(NIC):
                fillers.append((("qk", 3), qk_tile_job(3, qk, n)))

        for gp in range(4):
            for ic in range(NIC):
                drain_through(lambda key: key == ("qk", gp))
                drain_through(
                    lambda key: key[0] == "v" and key[1] <= 4 * ic + 3)
                for hh in range(2):
                    emit_unit(gp, hh, ic)
        while fillers:
            run_job(fillers.pop(0)[1])
        while pending_tail:
            emit_tail()

    _split_excess_waits(nc)
    return nc


_NC_CACHE = None


def _get_nc():
    global _NC_CACHE
    if _NC_CACHE is None:
        _NC_CACHE = _build_program()
    return _NC_CACHE


# ---------------------------------------------------------------------------
# Host-side sharding / unsharding
# ---------------------------------------------------------------------------
def _make_in_maps(x, W_qkv, b_qkv):
    scale = 1.0 / np.sqrt(np.float32(D))
    Wq, Wk, Wv = W_qkv[:, 0:D], W_qkv[:, D:2 * D], W_qkv[:, 2 * D:3 * D]
    bq, bk, bv = b_qkv[0:D], b_qkv[D:2 * D], b_qkv[2 * D:3 * D]
    in_maps = []
    for c in range(NCORES):
        b, g2 = divmod(c, 2)
        h0 = NH_CORE * g2  # first global head of this core
        xT = np.ascontiguousarray(x[b].T).astype(BF16)
        w_cols = []
        bqk_cols = []
        for gp in range(4):
            lo = (h0 + 2 * gp) * HD
            hi = lo + 2 * HD
            w_cols.append((Wq[:, lo:hi] * scale))
            w_cols.append(Wk[:, lo:hi])
            bqk_cols.append(bq[lo:hi] * scale)
            bqk_cols.append(bk[lo:hi])
        w_cols.append(Wv[:, h0 * HD:(h0 + NH_CORE) * HD])
        w = np.concatenate(w_cols, axis=1).astype(BF16)       # [1024, 1536]
        b_qk = np.stack(bqk_cols, axis=1).astype(np.float32)   # [128, 8]
        b_v = bv[h0 * HD:(h0 + NH_CORE) * HD].reshape(1, 512).astype(BF16)
        in_maps.append({"xT": xT, "w": w, "b_qk": b_qk, "b_v": b_v})
    return in_maps


def run(x, W_qkv, b_qkv, trace=False):
    """Run the distributed kernel; returns (out, BassKernelResults)."""
    nc = _get_nc()
    in_maps = _make_in_maps(np.asarray(x), np.asarray(W_qkv), np.asarray(b_qkv))
    res = run_bass_kernel_spmd(nc, in_maps, core_ids=list(range(NCORES)),
                               trace=trace)
    out = np.empty((4, T, D), dtype=np.float32)
    for c in range(NCORES):
        b, g2 = divmod(c, 2)
        out[b, :, g2 * 512:(g2 + 1) * 512] = res.results[c]["out"]
    return out, res


def kernel(x, W_qkv, b_qkv):
    out, _ = run(x, W_qkv, b_qkv, trace=False)
    return out


# revision 25
# speedup vs baseline: 1.1848x; 1.1848x over previous
"""Self-contained Trainium2 kernel for causal multi-head attention.

Module: x[4,2048,1024] -> QKV proj (16 heads, hd=64) -> causal softmax
(scale 1/sqrt(1024)) -> out [4,2048,1024].

Sharding: 8 cores = 4 batches x 2 head-groups (8 heads each). Each core is
fully independent (full seq per core, no collectives).

Per-core dataflow (transpose-free attention):
  - host pre-transposes x -> xT [1024,2048] and pre-packs W with the
    1/sqrt(d) scale folded into the Q columns; bf16 compute, fp32 PSUM.
  - QKV^T: qT/kT tiles [2*64, 2048] per head-pair via lhsT=W, rhs=xT;
    V in natural [seq, 64] layout via lhsT=xT, rhs=Wv (+bias via ones-matmul)
  - S^T[j,i] = kT_blk.T @ qT (K=64) per head; exp on ScalarE straight from
    PSUM; causal mask = multiply by 0/1 triangle on diag blocks only
  - O^T accum: lhsT=[V|1] (65 cols; col 64 gives softmax denominators free),
    rhs = exp(S^T); accumulate over j-tiles in PSUM
  - PE-transpose O^T -> O natural; DVE reciprocal + per-partition scale;
    DMA out [2048, 512] fp32 per core
"""

import sys
import types

import ml_dtypes
import numpy as np

# ---------------------------------------------------------------------------
# Environment shims (axon NTFF profile hook that this image's antenv lacks)
# ---------------------------------------------------------------------------
if "antenv.axon_hooks" not in sys.modules:
    try:
        import antenv

        try:
            from trn_agent_boot.trn_boot import _ntff_profile_via_ctypes

            _hook = _ntff_profile_via_ctypes("/opt/axon/libaxon_pjrt.so")
        except Exception:
            _hook = None
        _mod = types.ModuleType("antenv.axon_hooks")
        _mod.get_axon_ntff_profile_hook = lambda: _hook
        _mod.set_axon_ntff_profile_hook = lambda h: None
        sys.modules["antenv.axon_hooks"] = _mod
        antenv.axon_hooks = _mod
    except ImportError:
        pass

import concourse.bass as bass
import concourse.mybir as mybir
import concourse.tile as tile
from concourse.bass_utils import run_bass_kernel_spmd
from concourse.masks import make_identity

BF16 = ml_dtypes.bfloat16

T = 2048          # sequence length
D = 1024          # model dim
NH_CORE = 8       # heads per core
HD = 64           # head dim
NCORES = 8
NKC = D // 128    # contraction chunks (8)
NIC = T // 512    # 512-wide i chunks (4)
NJT = T // 128    # 128-wide j tiles (16)
F32 = mybir.dt.float32
BF = mybir.dt.bfloat16


# ---------------------------------------------------------------------------
# walrus workaround: split instructions with >1 semaphore wait into chained
# NoOps (this container's walrus rejects >1 sync-wait per instruction).
# ---------------------------------------------------------------------------
def _split_excess_waits(nc, max_waits=1):
    n_split = 0
    for f in nc.m.functions:
        for blk in f.blocks:
            new_insts = []
            for inst in blk.instructions:
                si = inst.sync_info
                if si is None or si.on_wait is None or len(si.on_wait) <= max_waits:
                    new_insts.append(inst)
                    continue
                waits = list(si.on_wait)
                movable = [w for w in waits if w.wait_mode == "sem-ge-imm"]
                fixed = [w for w in waits if w.wait_mode != "sem-ge-imm"]
                keep = max_waits - len(fixed)
                assert keep >= 0, f"{inst.name}: too many non-ge waits"
                kept = fixed + (movable[:keep] if keep > 0 else [])
                rest = movable[keep:] if keep > 0 else movable
                for i in range(0, len(rest), max_waits):
                    chunk = rest[i:i + max_waits]
                    n_split += 1
                    new_insts.append(mybir.InstNoOp(
                        name=f"I-waitsplit-{n_split}",
                        engine=inst.engine,
                        ins=[], outs=[],
                        sync_info=mybir.SyncInfo(on_wait=list(chunk), on_update=[]),
                        bass_nofuse=True,
                    ))
                inst.sync_info = mybir.SyncInfo(
                    on_wait=kept, on_update=list(si.on_update or []))
                new_insts.append(inst)
            blk.instructions = new_insts
    return n_split


# ---------------------------------------------------------------------------
# Device program
# ---------------------------------------------------------------------------
def _build_program():
    from contextlib import ExitStack

    nc = bass.Bass(target_bir_lowering=False, debug=False)
    xT_ext = nc.declare_dram_parameter("xT", [D, T], BF, isOutput=False)
    w_ext = nc.declare_dram_parameter("w", [D, 1536], BF, isOutput=False)
    bqk_ext = nc.declare_dram_parameter("b_qk", [128, 8], F32, isOutput=False)
    bv_ext = nc.declare_dram_parameter("b_v", [1, 512], BF, isOutput=False)
    out_ext = nc.declare_dram_parameter("out", [T, NH_CORE * HD], F32, isOutput=True)

    with tile.TileContext(nc) as tc, ExitStack() as ctx:
        const = ctx.enter_context(tc.tile_pool(name="const", bufs=1))
        # PSUM: "mm" slots are 2 banks ([128,1024] f32); fl/acc slots 1 bank
        psum_mm = ctx.enter_context(tc.tile_pool(name="psum_mm", bufs=2, space="PSUM"))
        psum_fl = ctx.enter_context(tc.tile_pool(name="psum_fl", bufs=2, space="PSUM"))
        psum_acc = ctx.enter_context(tc.tile_pool(name="psum_acc", bufs=2, space="PSUM"))
        p_pool = ctx.enter_context(tc.tile_pool(name="p_pool", bufs=6))
        ot_pool = ctx.enter_context(tc.tile_pool(name="ot_pool", bufs=3))
        o_pool = ctx.enter_context(tc.tile_pool(name="o_pool", bufs=3))
        r_pool = ctx.enter_context(tc.tile_pool(name="r_pool", bufs=2))

        # persistent SBUF tensors (split finely so Tile's per-tile dependency
        # tracking doesn't serialize phases)
        identb_sb = const.tile([128, 128], BF)
        xT_sb = [const.tile([128, T], BF, tag=f"xT{kc}", name=f"xT{kc}") for kc in range(NKC)]
        w_sb = [const.tile([128, 1536], BF, tag=f"w{kc}", name=f"w{kc}") for kc in range(NKC)]
        qt_sb = [const.tile([128, T], BF, tag=f"qt{gp}", name=f"qt{gp}") for gp in range(4)]
        kt_sb = [const.tile([128, T], BF, tag=f"kt{gp}", name=f"kt{gp}") for gp in range(4)]
        v_sb = [const.tile([128, NH_CORE * 65], BF, tag=f"v{jt}", name=f"v{jt}") for jt in range(NJT)]
        bqk_sb = const.tile([128, 8], F32)
        bv_sb = const.tile([1, 512], BF)
        ones_sb = const.tile([1, 128], BF)
        mask_sb = const.tile([128, 128], BF)

        # --- setup ---
        for kc in range(NKC):
            nc.gpsimd.dma_start(xT_sb[kc][:, :], xT_ext[kc * 128:(kc + 1) * 128, :])
            nc.gpsimd.dma_start(w_sb[kc][:, :], w_ext[kc * 128:(kc + 1) * 128, :])
        nc.gpsimd.dma_start(bqk_sb[:, :], bqk_ext[:, :])
        nc.gpsimd.dma_start(bv_sb[:, :], bv_ext[:, :])
        nc.vector.memset(ones_sb[:, :], 1.0)
        make_identity(nc, identb_sb[:, :])
        # causal 0/1 triangle (diagonal 128-col block): keep 1.0 where p <= f
        nc.gpsimd.memset(mask_sb[:, :], 1.0)
        nc.gpsimd.affine_select(
            out=mask_sb[:, :], in_=mask_sb[:, :],
            compare_op=mybir.AluOpType.is_ge, fill=0.0,
            base=0, pattern=[[1, 128]], channel_multiplier=-1,
        )
        for jt in range(NJT):
            nc.vector.memset(
                v_sb[jt][:, :].rearrange("p (h c) -> p h c", c=65)[:, :, 64:65], 1.0)

        def qk_tile_job(gp, qk, n):
            t_idx = 2 * gp + qk
            dest = qt_sb[gp] if qk == 0 else kt_sb[gp]
            ps = psum_fl.tile([128, 512], F32, tag="fl", name=f"flq{gp}_{qk}_{n}")
            for kc in range(NKC):
                nc.tensor.matmul(
                    ps[:, :],
                    lhsT=w_sb[kc][:, t_idx * 128:(t_idx + 1) * 128],
                    rhs=xT_sb[kc][:, n * 512:(n + 1) * 512],
                    start=(kc == 0), stop=(kc == NKC - 1),
                )
                yield
            nc.vector.tensor_scalar_add(
                dest[:, n * 512:(n + 1) * 512], ps[:, :],
                bqk_sb[:, t_idx:t_idx + 1],
            )
            yield

        def v_tile_job(st):
            ps = psum_fl.tile([128, 512], F32, tag="fl", name=f"flv{st}")
            nc.tensor.matmul(ps[:, :], lhsT=ones_sb[:, :], rhs=bv_sb[:, :],
                             start=True, stop=False)
            for kc in range(NKC):
                nc.tensor.matmul(
                    ps[:, :],
                    lhsT=xT_sb[kc][:, st * 128:(st + 1) * 128],
                    rhs=w_sb[kc][:, 1024:1536],
                    start=False, stop=(kc == NKC - 1),
                )
                yield
            nc.vector.tensor_copy(
                v_sb[st][:, :].rearrange("p (h c) -> p h c", c=65)[:, :, 0:64],
                ps[:, :].rearrange("p (h c) -> p h c", c=64),
            )
            yield

        est = {"pe": 0.0, "act": 0.0}

        def run_job(gen):
            for _ in gen:
                est["pe"] += 215.0

        # filler queue: [(key, generator)] pumped into phase-2 PE bubbles
        fillers = []

        def pump_one():
            while fillers:
                try:
                    next(fillers[0][1])
                    est["pe"] += 215.0
                    return True
                except StopIteration:
                    fillers.pop(0)
            return False

        def pump_balance():
            while fillers and est["pe"] < est["act"] + 2000.0:
                if not pump_one():
                    return

        def drain_through(pred):
            """Run filler jobs (FIFO) until every job matching pred is gone."""
            while any(pred(key) for key, _ in fillers):
                run_job(fillers[0][1])
                fillers.pop(0)

        pending_tail = []

        def emit_tail():
            if not pending_tail:
                return
            h, ic, ot_s = pending_tail.pop(0)
            tr_ps = psum_fl.tile([128, 264], BF, tag="fl")
            for blk in range(4):
                nc.tensor.transpose(
                    tr_ps[:, blk * 66:blk * 66 + 65],
                    ot_s[0:65, blk * 128:(blk + 1) * 128],
                    identb_sb[0:65, 0:65],
                )
            rc = r_pool.tile([128, 4], F32)
            nc.vector.reciprocal(
                rc[:, :],
                tr_ps[:, 0:264].rearrange("p (b c) -> p b c", c=66)[:, :, 64:65],
            )
            o_s = o_pool.tile([128, 256], F32)
            for blk in range(4):
                nc.vector.tensor_scalar_mul(
                    o_s[:, blk * 64:(blk + 1) * 64],
                    tr_ps[:, blk * 66:blk * 66 + 64],
                    rc[:, blk:blk + 1],
                )
            nc.gpsimd.dma_start(
                out_ext[ic * 512:(ic + 1) * 512, h * 64:(h + 1) * 64]
                .rearrange("(blk p) c -> p blk c", p=128),
                o_s[:, :].rearrange("p (blk c) -> p blk c", c=64),
            )

        def emit_unit(gp, hh, ic):
            h = 2 * gp + hh
            po = 64 * hh
            njt = 4 * ic + 4
            acc = psum_acc.tile([65, 512], F32, tag="acc")
            for jta in range(0, njt, 2):
                jtb = jta + 1
                ra = jta - 4 * ic
                rb = jtb - 4 * ic
                f0a = 128 * ra if ra >= 0 else 0
                f0b = 128 * rb if rb >= 0 else 0
                st2 = psum_mm.tile([128, 1024], F32, tag="mm")
                nc.tensor.matmul(
                    st2[:, f0a:512],
                    lhsT=kt_sb[gp][po:po + 64, jta * 128:(jta + 1) * 128],
                    rhs=qt_sb[gp][po:po + 64, ic * 512 + f0a: ic * 512 + 512],
                    start=True, stop=True,
                )
                nc.tensor.matmul(
                    st2[:, 512 + f0b:1024],
                    lhsT=kt_sb[gp][po:po + 64, jtb * 128:(jtb + 1) * 128],
                    rhs=qt_sb[gp][po:po + 64, ic * 512 + f0b: ic * 512 + 512],
                    start=True, stop=True,
                )
                pump_one()
                p_t = p_pool.tile([128, 1024], BF, tag="pt")
                if ra >= 0:
                    # diagonal pair: exp only the written regions
                    nc.scalar.activation(
                        p_t[:, f0a:512], st2[:, f0a:512],
                        mybir.ActivationFunctionType.Exp)
                    nc.scalar.activation(
                        p_t[:, 512 + f0b:1024], st2[:, 512 + f0b:1024],
                        mybir.ActivationFunctionType.Exp)
                    est["act"] += (172 + 512 - f0a) / 1.2 + (172 + 512 - f0b) / 1.2
                    nc.vector.tensor_mul(
                        p_t[:, f0a:f0a + 128], p_t[:, f0a:f0a + 128], mask_sb[:, :])
                    nc.vector.tensor_mul(
                        p_t[:, 512 + f0b:512 + f0b + 128],
                        p_t[:, 512 + f0b:512 + f0b + 128], mask_sb[:, :])
                else:
                    nc.scalar.activation(
                        p_t[:, :], st2[:, :], mybir.ActivationFunctionType.Exp)
                    est["act"] += (172 + 1024) / 1.2
                nc.tensor.matmul(
                    acc[0:65, f0a:512],
                    lhsT=v_sb[jta][:, h * 65:(h + 1) * 65],
                    rhs=p_t[:, f0a:512],
                    start=(jta == 0), stop=False,
                )
                nc.tensor.matmul(
                    acc[0:65, f0b:512],
                    lhsT=v_sb[jtb][:, h * 65:(h + 1) * 65],
                    rhs=p_t[:, 512 + f0b:1024],
                    start=False, stop=(jtb == njt - 1),
                )
                pump_one()
            ot_s = const.tile([65, 512], BF, tag=f"ot{h}_{ic}", name=f"ot{h}_{ic}")
            nc.vector.tensor_copy(ot_s[:, :], acc[:, :])
            pending_tail.append((h, ic, ot_s))

        # --- emission: qk(pair0) + V(0-3) upfront; the rest becomes filler
        # work pumped into phase-2 PE bubbles (keeps TensorE dense -> HAM warm)
        for qk in range(2):
            for n in range(NIC):
                run_job(qk_tile_job(0, qk, n))
        for st in range(4):
            run_job(v_tile_job(st))
        for st in range(4, 8):
            fillers.append((("v", st), v_tile_job(st)))
        for qk in range(2):
            for n in range(NIC):
                fillers.append((("qk", 1), qk_tile_job(1, qk, n)))
        for st in range(8, 12):
            fillers.append((("v", st), v_tile_job(st)))
        for qk in range(2):
            for n in range(NIC):
                fillers.append((("qk", 2), qk_tile_job(2, qk, n)))
        for st in range(12, 16):
            fillers.append((("v", st), v_tile_job(st)))
        for qk in range(2):
            for n in range(NIC):
                fillers.append((("qk", 3), qk_tile_job(3, qk, n)))

        for gp in range(4):
            for ic in range(NIC):
                drain_through(lambda key: key == ("qk", gp))
                drain_through(
                    lambda key: key[0] == "v" and key[1] <= 4 * ic + 3)
                for hh in range(2):
                    emit_unit(gp, hh, ic)
        while fillers:
            run_job(fillers.pop(0)[1])
        while pending_tail:
            emit_tail()

    _split_excess_waits(nc)
    return nc


_NC_CACHE = None


def _get_nc():
    global _NC_CACHE
    if _NC_CACHE is None:
        _NC_CACHE = _build_program()
    return _NC_CACHE


# ---------------------------------------------------------------------------
# Host-side sharding / unsharding
# ---------------------------------------------------------------------------
def _make_in_maps(x, W_qkv, b_qkv):
    scale = 1.0 / np.sqrt(np.float32(D))
    Wq, Wk, Wv = W_qkv[:, 0:D], W_qkv[:, D:2 * D], W_qkv[:, 2 * D:3 * D]
    bq, bk, bv = b_qkv[0:D], b_qkv[D:2 * D], b_qkv[2 * D:3 * D]
    in_maps = []
    for c in range(NCORES):
        b, g2 = divmod(c, 2)
        h0 = NH_CORE * g2  # first global head of this core
        xT = np.ascontiguousarray(x[b].T).astype(BF16)
        w_cols = []
        bqk_cols = []
        for gp in range(4):
            lo = (h0 + 2 * gp) * HD
            hi = lo + 2 * HD
            w_cols.append((Wq[:, lo:hi] * scale))
            w_cols.append(Wk[:, lo:hi])
            bqk_cols.append(bq[lo:hi] * scale)
            bqk_cols.append(bk[lo:hi])
        w_cols.append(Wv[:, h0 * HD:(h0 + NH_CORE) * HD])
        w = np.concatenate(w_cols, axis=1).astype(BF16)       # [1024, 1536]
        b_qk = np.stack(bqk_cols, axis=1).astype(np.float32)   # [128, 8]
        b_v = bv[h0 * HD:(h0 + NH_CORE) * HD].reshape(1, 512).astype(BF16)
        in_maps.append({"xT": xT, "w": w, "b_qk": b_qk, "b_v": b_v})
    return in_maps


def run(x, W_qkv, b_qkv, trace=False):
    """Run the distributed kernel; returns (out, BassKernelResults)."""
    nc = _get_nc()
    in_maps = _make_in_maps(np.asarray(x), np.asarray(W_qkv), np.asarray(b_qkv))
    res = run_bass_kernel_spmd(nc, in_maps, core_ids=list(range(NCORES)),
                               trace=trace)
    out = np.empty((4, T, D), dtype=np.float32)
    for c in range(NCORES):
        b, g2 = divmod(c, 2)
        out[b, :, g2 * 512:(g2 + 1) * 512] = res.results[c]["out"]
    return out, res


def kernel(x, W_qkv, b_qkv):
    out, _ = run(x, W_qkv, b_qkv, trace=False)
    return out


# revision 27
# speedup vs baseline: 1.4331x; 1.2096x over previous
"""Self-contained Trainium2 kernel for causal multi-head attention.

Module: x[4,2048,1024] -> QKV proj (16 heads, hd=64) -> causal softmax
(scale 1/sqrt(1024)) -> out [4,2048,1024].

Sharding: 8 cores = 4 batches x 2 head-groups (8 heads each). Each core is
fully independent (full seq per core, no collectives).

Per-core dataflow (transpose-free attention):
  - host pre-transposes x -> xT [1024,2048] and pre-packs W with the
    1/sqrt(d) scale folded into the Q columns; bf16 compute, fp32 PSUM.
  - QKV^T: qT/kT tiles [2*64, 2048] per head-pair via lhsT=W, rhs=xT;
    V in natural [seq, 64] layout via lhsT=xT, rhs=Wv (+bias via ones-matmul)
  - S^T[j,i] = kT_blk.T @ qT (K=64) per head; exp on ScalarE straight from
    PSUM; causal mask = multiply by 0/1 triangle on diag blocks only
  - O^T accum: lhsT=[V|1] (65 cols; col 64 gives softmax denominators free),
    rhs = exp(S^T); accumulate over j-tiles in PSUM
  - PE-transpose O^T -> O natural; DVE reciprocal + per-partition scale;
    DMA out [2048, 512] fp32 per core
"""

import sys
import types

import ml_dtypes
import numpy as np

# ---------------------------------------------------------------------------
# Environment shims (axon NTFF profile hook that this image's antenv lacks)
# ---------------------------------------------------------------------------
if "antenv.axon_hooks" not in sys.modules:
    try:
        import antenv

        try:
            from trn_agent_boot.trn_boot import _ntff_profile_via_ctypes

            _hook = _ntff_profile_via_ctypes("/opt/axon/libaxon_pjrt.so")
        except Exception:
            _hook = None
        _mod = types.ModuleType("antenv.axon_hooks")
        _mod.get_axon_ntff_profile_hook = lambda: _hook
        _mod.set_axon_ntff_profile_hook = lambda h: None
        sys.modules["antenv.axon_hooks"] = _mod
        antenv.axon_hooks = _mod
    except ImportError:
        pass

import concourse.bass as bass
import concourse.mybir as mybir
import concourse.tile as tile
from concourse.bass_utils import run_bass_kernel_spmd
from concourse.masks import make_identity

BF16 = ml_dtypes.bfloat16

T = 2048          # sequence length
D = 1024          # model dim
NH_CORE = 8       # heads per core
HD = 64           # head dim
NCORES = 8
NKC = D // 128    # contraction chunks (8)
NIC = T // 512    # 512-wide i chunks (4)
NJT = T // 128    # 128-wide j tiles (16)
F32 = mybir.dt.float32
BF = mybir.dt.bfloat16


# ---------------------------------------------------------------------------
# walrus workaround: split instructions with >1 semaphore wait into chained
# NoOps (this container's walrus rejects >1 sync-wait per instruction).
# ---------------------------------------------------------------------------
def _split_excess_waits(nc, max_waits=1):
    n_split = 0
    for f in nc.m.functions:
        for blk in f.blocks:
            new_insts = []
            for inst in blk.instructions:
                si = inst.sync_info
                if si is None or si.on_wait is None or len(si.on_wait) <= max_waits:
                    new_insts.append(inst)
                    continue
                waits = list(si.on_wait)
                movable = [w for w in waits if w.wait_mode == "sem-ge-imm"]
                fixed = [w for w in waits if w.wait_mode != "sem-ge-imm"]
                keep = max_waits - len(fixed)
                assert keep >= 0, f"{inst.name}: too many non-ge waits"
                kept = fixed + (movable[:keep] if keep > 0 else [])
                rest = movable[keep:] if keep > 0 else movable
                for i in range(0, len(rest), max_waits):
                    chunk = rest[i:i + max_waits]
                    n_split += 1
                    new_insts.append(mybir.InstNoOp(
                        name=f"I-waitsplit-{n_split}",
                        engine=inst.engine,
                        ins=[], outs=[],
                        sync_info=mybir.SyncInfo(on_wait=list(chunk), on_update=[]),
                        bass_nofuse=True,
                    ))
                inst.sync_info = mybir.SyncInfo(
                    on_wait=kept, on_update=list(si.on_update or []))
                new_insts.append(inst)
            blk.instructions = new_insts
    return n_split


# ---------------------------------------------------------------------------
# Device program
# ---------------------------------------------------------------------------
def _build_program():
    from contextlib import ExitStack

    nc = bass.Bass(target_bir_lowering=False, debug=False)
    xT_ext = nc.declare_dram_parameter("xT", [D, T], BF, isOutput=False)
    w_ext = nc.declare_dram_parameter("w", [D, 1536], BF, isOutput=False)
    bqk_ext = nc.declare_dram_parameter("b_qk", [128, 8], F32, isOutput=False)
    bv_ext = nc.declare_dram_parameter("b_v", [1, 512], BF, isOutput=False)
    out_ext = nc.declare_dram_parameter("out", [T, NH_CORE * HD], F32, isOutput=True)

    with tile.TileContext(nc) as tc, ExitStack() as ctx:
        const = ctx.enter_context(tc.tile_pool(name="const", bufs=1))
        # PSUM: "mm" slots are 2 banks ([128,1024] f32); fl/acc slots 1 bank
        psum_mm = ctx.enter_context(tc.tile_pool(name="psum_mm", bufs=2, space="PSUM"))
        psum_fl = ctx.enter_context(tc.tile_pool(name="psum_fl", bufs=2, space="PSUM"))
        psum_acc = ctx.enter_context(tc.tile_pool(name="psum_acc", bufs=2, space="PSUM"))
        p_pool = ctx.enter_context(tc.tile_pool(name="p_pool", bufs=6))
        ot_pool = ctx.enter_context(tc.tile_pool(name="ot_pool", bufs=3))
        o_pool = ctx.enter_context(tc.tile_pool(name="o_pool", bufs=3))
        r_pool = ctx.enter_context(tc.tile_pool(name="r_pool", bufs=2))

        # persistent SBUF tensors (split finely so Tile's per-tile dependency
        # tracking doesn't serialize phases)
        identb_sb = const.tile([128, 128], BF)
        xT_sb = [const.tile([128, T], BF, tag=f"xT{kc}", name=f"xT{kc}") for kc in range(NKC)]
        w_sb = [const.tile([128, 1536], BF, tag=f"w{kc}", name=f"w{kc}") for kc in range(NKC)]
        qt_sb = [const.tile([128, T], BF, tag=f"qt{gp}", name=f"qt{gp}") for gp in range(4)]
        kt_sb = [const.tile([128, T], BF, tag=f"kt{gp}", name=f"kt{gp}") for gp in range(4)]
        v_sb = [const.tile([128, NH_CORE * 65], BF, tag=f"v{jt}", name=f"v{jt}") for jt in range(NJT)]
        bqk_sb = const.tile([128, 8], F32)
        bv_sb = const.tile([1, 512], BF)
        ones_sb = const.tile([1, 128], BF)
        mask_sb = const.tile([128, 128], BF)

        # --- setup ---
        for kc in range(NKC):
            nc.gpsimd.dma_start(xT_sb[kc][:, :], xT_ext[kc * 128:(kc + 1) * 128, :])
            nc.gpsimd.dma_start(w_sb[kc][:, :], w_ext[kc * 128:(kc + 1) * 128, :])
        nc.gpsimd.dma_start(bqk_sb[:, :], bqk_ext[:, :])
        nc.gpsimd.dma_start(bv_sb[:, :], bv_ext[:, :])
        nc.vector.memset(ones_sb[:, :], 1.0)
        make_identity(nc, identb_sb[:, :])
        # causal 0/1 triangle (diagonal 128-col block): keep 1.0 where p <= f
        nc.gpsimd.memset(mask_sb[:, :], 1.0)
        nc.gpsimd.affine_select(
            out=mask_sb[:, :], in_=mask_sb[:, :],
            compare_op=mybir.AluOpType.is_ge, fill=0.0,
            base=0, pattern=[[1, 128]], channel_multiplier=-1,
        )
        for jt in range(NJT):
            nc.vector.memset(
                v_sb[jt][:, :].rearrange("p (h c) -> p h c", c=65)[:, :, 64:65], 1.0)

        def qk_tile_job(gp, qk, n):
            t_idx = 2 * gp + qk
            dest = qt_sb[gp] if qk == 0 else kt_sb[gp]
            ps = psum_fl.tile([128, 512], F32, tag="fl", name=f"flq{gp}_{qk}_{n}")
            for kc in range(NKC):
                nc.tensor.matmul(
                    ps[:, :],
                    lhsT=w_sb[kc][:, t_idx * 128:(t_idx + 1) * 128],
                    rhs=xT_sb[kc][:, n * 512:(n + 1) * 512],
                    start=(kc == 0), stop=(kc == NKC - 1),
                )
                yield
            nc.vector.tensor_scalar_add(
                dest[:, n * 512:(n + 1) * 512], ps[:, :],
                bqk_sb[:, t_idx:t_idx + 1],
            )
            yield

        def v_tile_job(st):
            ps = psum_fl.tile([128, 512], F32, tag="fl", name=f"flv{st}")
            nc.tensor.matmul(ps[:, :], lhsT=ones_sb[:, :], rhs=bv_sb[:, :],
                             start=True, stop=False)
            for kc in range(NKC):
                nc.tensor.matmul(
                    ps[:, :],
                    lhsT=xT_sb[kc][:, st * 128:(st + 1) * 128],
                    rhs=w_sb[kc][:, 1024:1536],
                    start=False, stop=(kc == NKC - 1),
                )
                yield
            nc.vector.tensor_copy(
                v_sb[st][:, :].rearrange("p (h c) -> p h c", c=65)[:, :, 0:64],
                ps[:, :].rearrange("p (h c) -> p h c", c=64),
            )
            yield

        est = {"pe": 0.0, "act": 0.0}

        def run_job(gen):
            for _ in gen:
                est["pe"] += 215.0

        # filler queue: [(key, generator)] pumped into phase-2 PE bubbles
        fillers = []

        def pump_one():
            while fillers:
                try:
                    next(fillers[0][1])
                    est["pe"] += 215.0
                    return True
                except StopIteration:
                    fillers.pop(0)
            return False

        def pump_balance():
            while fillers and est["pe"] < est["act"] + 2000.0:
                if not pump_one():
                    return

        def drain_through(pred):
            """Run filler jobs (FIFO) until every job matching pred is gone."""
            while any(pred(key) for key, _ in fillers):
                run_job(fillers[0][1])
                fillers.pop(0)

        pending_tail = []

        def emit_tail():
            if not pending_tail:
                return
            h, ic, ot_s = pending_tail.pop(0)
            tr_ps = psum_fl.tile([128, 264], BF, tag="fl")
            for blk in range(4):
                nc.tensor.transpose(
                    tr_ps[:, blk * 66:blk * 66 + 65],
                    ot_s[0:65, blk * 128:(blk + 1) * 128],
                    identb_sb[0:65, 0:65],
                )
            rc = r_pool.tile([128, 4], F32)
            nc.vector.reciprocal(
                rc[:, :],
                tr_ps[:, 0:264].rearrange("p (b c) -> p b c", c=66)[:, :, 64:65],
            )
            o_s = o_pool.tile([128, 256], F32)
            for blk in range(4):
                nc.vector.tensor_scalar_mul(
                    o_s[:, blk * 64:(blk + 1) * 64],
                    tr_ps[:, blk * 66:blk * 66 + 64],
                    rc[:, blk:blk + 1],
                )
            nc.gpsimd.dma_start(
                out_ext[ic * 512:(ic + 1) * 512, h * 64:(h + 1) * 64]
                .rearrange("(blk p) c -> p blk c", p=128),
                o_s[:, :].rearrange("p (blk c) -> p blk c", c=64),
            )

        def emit_unit(gp, ic):
            # both heads of the pair processed per round; their K=64 S^T
            # matmuls land on row-tiles T0/T8 (base partitions 0/64) and run
            # concurrently in the 64x128 tiled array mode
            h0 = 2 * gp
            h1 = 2 * gp + 1
            njt = 4 * ic + 4
            acc0 = psum_acc.tile([65, 512], F32, tag="acc")
            acc1 = psum_acc.tile([65, 512], F32, tag="acc")
            for jt in range(njt):
                r = jt - 4 * ic
                f0 = 128 * r if r >= 0 else 0
                st2 = psum_mm.tile([128, 1024], F32, tag="mm")
                nc.tensor.matmul(
                    st2[:, f0:512],
                    lhsT=kt_sb[gp][0:64, jt * 128:(jt + 1) * 128],
                    rhs=qt_sb[gp][0:64, ic * 512 + f0: ic * 512 + 512],
                    start=True, stop=True,
                )
                nc.tensor.matmul(
                    st2[:, 512 + f0:1024],
                    lhsT=kt_sb[gp][64:128, jt * 128:(jt + 1) * 128],
                    rhs=qt_sb[gp][64:128, ic * 512 + f0: ic * 512 + 512],
                    start=True, stop=True,
                )
                pump_one()
                p_t = p_pool.tile([128, 1024], BF, tag="pt")
                if r >= 0:
                    # diagonal tile: exp only the written regions
                    nc.scalar.activation(
                        p_t[:, f0:512], st2[:, f0:512],
                        mybir.ActivationFunctionType.Exp)
                    nc.scalar.activation(
                        p_t[:, 512 + f0:1024], st2[:, 512 + f0:1024],
                        mybir.ActivationFunctionType.Exp)
                    est["act"] += 2 * (172 + 512 - f0) / 1.2
                    nc.vector.tensor_mul(
                        p_t[:, f0:f0 + 128], p_t[:, f0:f0 + 128], mask_sb[:, :])
                    nc.vector.tensor_mul(
                        p_t[:, 512 + f0:512 + f0 + 128],
                        p_t[:, 512 + f0:512 + f0 + 128], mask_sb[:, :])
                else:
                    nc.scalar.activation(
                        p_t[:, :], st2[:, :], mybir.ActivationFunctionType.Exp)
                    est["act"] += (172 + 1024) / 1.2
                nc.tensor.matmul(
                    acc0[0:65, f0:512],
                    lhsT=v_sb[jt][:, h0 * 65:(h0 + 1) * 65],
                    rhs=p_t[:, f0:512],
                    start=(jt == 0), stop=(jt == njt - 1),
                )
                nc.tensor.matmul(
                    acc1[0:65, f0:512],
                    lhsT=v_sb[jt][:, h1 * 65:(h1 + 1) * 65],
                    rhs=p_t[:, 512 + f0:1024],
                    start=(jt == 0), stop=(jt == njt - 1),
                )
                pump_one()
            for h, acc in ((h0, acc0), (h1, acc1)):
                ot_s = const.tile([65, 512], BF, tag=f"ot{h}_{ic}", name=f"ot{h}_{ic}")
                nc.vector.tensor_copy(ot_s[:, :], acc[:, :])
                pending_tail.append((h, ic, ot_s))

        # --- emission: qk(pair0) + V(0-3) upfront; the rest becomes filler
        # work pumped into phase-2 PE bubbles (keeps TensorE dense -> HAM warm)
        for qk in range(2):
            for n in range(NIC):
                run_job(qk_tile_job(0, qk, n))
        for st in range(4):
            run_job(v_tile_job(st))
        for st in range(4, 8):
            fillers.append((("v", st), v_tile_job(st)))
        for qk in range(2):
            for n in range(NIC):
                fillers.append((("qk", 1), qk_tile_job(1, qk, n)))
        for st in range(8, 12):
            fillers.append((("v", st), v_tile_job(st)))
        for qk in range(2):
            for n in range(NIC):
                fillers.append((("qk", 2), qk_tile_job(2, qk, n)))
        for st in range(12, 16):
            fillers.append((("v", st), v_tile_job(st)))
        for qk in range(2):
            for n in range(NIC):
                fillers.append((("qk", 3), qk_tile_job(3, qk, n)))

        for gp in range(4):
            for ic in range(NIC):
                drain_through(lambda key: key == ("qk", gp))
                drain_through(
                    lambda key: key[0] == "v" and key[1] <= 4 * ic + 3)
                emit_unit(gp, ic)
        while fillers:
            run_job(fillers.pop(0)[1])
        while pending_tail:
            emit_tail()

    _split_excess_waits(nc)
    return nc


_NC_CACHE = None


def _get_nc():
    global _NC_CACHE
    if _NC_CACHE is None:
        _NC_CACHE = _build_program()
    return _NC_CACHE


# ---------------------------------------------------------------------------
# Host-side sharding / unsharding
# ---------------------------------------------------------------------------
def _make_in_maps(x, W_qkv, b_qkv):
    scale = 1.0 / np.sqrt(np.float32(D))
    Wq, Wk, Wv = W_qkv[:, 0:D], W_qkv[:, D:2 * D], W_qkv[:, 2 * D:3 * D]
    bq, bk, bv = b_qkv[0:D], b_qkv[D:2 * D], b_qkv[2 * D:3 * D]
    in_maps = []
    for c in range(NCORES):
        b, g2 = divmod(c, 2)
        h0 = NH_CORE * g2  # first global head of this core
        xT = np.ascontiguousarray(x[b].T).astype(BF16)
        w_cols = []
        bqk_cols = []
        for gp in range(4):
            lo = (h0 + 2 * gp) * HD
            hi = lo + 2 * HD
            w_cols.append((Wq[:, lo:hi] * scale))
            w_cols.append(Wk[:, lo:hi])
            bqk_cols.append(bq[lo:hi] * scale)
            bqk_cols.append(bk[lo:hi])
        w_cols.append(Wv[:, h0 * HD:(h0 + NH_CORE) * HD])
        w = np.concatenate(w_cols, axis=1).astype(BF16)       # [1024, 1536]
        b_qk = np.stack(bqk_cols, axis=1).astype(np.float32)   # [128, 8]
        b_v = bv[h0 * HD:(h0 + NH_CORE) * HD].reshape(1, 512).astype(BF16)
        in_maps.append({"xT": xT, "w": w, "b_qk": b_qk, "b_v": b_v})
    return in_maps


def run(x, W_qkv, b_qkv, trace=False):
    """Run the distributed kernel; returns (out, BassKernelResults)."""
    nc = _get_nc()
    in_maps = _make_in_maps(np.asarray(x), np.asarray(W_qkv), np.asarray(b_qkv))
    res = run_bass_kernel_spmd(nc, in_maps, core_ids=list(range(NCORES)),
                               trace=trace)
    out = np.empty((4, T, D), dtype=np.float32)
    for c in range(NCORES):
        b, g2 = divmod(c, 2)
        out[b, :, g2 * 512:(g2 + 1) * 512] = res.results[c]["out"]
    return out, res


def kernel(x, W_qkv, b_qkv):
    out, _ = run(x, W_qkv, b_qkv, trace=False)
    return out


# revision 28
# speedup vs baseline: 1.4565x; 1.0163x over previous
"""Self-contained Trainium2 kernel for causal multi-head attention.

Module: x[4,2048,1024] -> QKV proj (16 heads, hd=64) -> causal softmax
(scale 1/sqrt(1024)) -> out [4,2048,1024].

Sharding: 8 cores = 4 batches x 2 head-groups (8 heads each). Each core is
fully independent (full seq per core, no collectives).

Per-core dataflow (transpose-free attention):
  - host pre-transposes x -> xT [1024,2048] and pre-packs W with the
    1/sqrt(d) scale folded into the Q columns; bf16 compute, fp32 PSUM.
  - QKV^T: qT/kT tiles [2*64, 2048] per head-pair via lhsT=W, rhs=xT;
    V in natural [seq, 64] layout via lhsT=xT, rhs=Wv (+bias via ones-matmul)
  - S^T[j,i] = kT_blk.T @ qT (K=64) per head; exp on ScalarE straight from
    PSUM; causal mask = multiply by 0/1 triangle on diag blocks only
  - O^T accum: lhsT=[V|1] (65 cols; col 64 gives softmax denominators free),
    rhs = exp(S^T); accumulate over j-tiles in PSUM
  - PE-transpose O^T -> O natural; DVE reciprocal + per-partition scale;
    DMA out [2048, 512] fp32 per core
"""

import sys
import types

import ml_dtypes
import numpy as np

# ---------------------------------------------------------------------------
# Environment shims (axon NTFF profile hook that this image's antenv lacks)
# ---------------------------------------------------------------------------
if "antenv.axon_hooks" not in sys.modules:
    try:
        import antenv

        try:
            from trn_agent_boot.trn_boot import _ntff_profile_via_ctypes

            _hook = _ntff_profile_via_ctypes("/opt/axon/libaxon_pjrt.so")
        except Exception:
            _hook = None
        _mod = types.ModuleType("antenv.axon_hooks")
        _mod.get_axon_ntff_profile_hook = lambda: _hook
        _mod.set_axon_ntff_profile_hook = lambda h: None
        sys.modules["antenv.axon_hooks"] = _mod
        antenv.axon_hooks = _mod
    except ImportError:
        pass

import concourse.bass as bass
import concourse.mybir as mybir
import concourse.tile as tile
from concourse.bass_utils import run_bass_kernel_spmd
from concourse.masks import make_identity

BF16 = ml_dtypes.bfloat16

T = 2048          # sequence length
D = 1024          # model dim
NH_CORE = 8       # heads per core
HD = 64           # head dim
NCORES = 8
NKC = D // 128    # contraction chunks (8)
NIC = T // 512    # 512-wide i chunks (4)
NJT = T // 128    # 128-wide j tiles (16)
F32 = mybir.dt.float32
BF = mybir.dt.bfloat16


# ---------------------------------------------------------------------------
# walrus workaround: split instructions with >1 semaphore wait into chained
# NoOps (this container's walrus rejects >1 sync-wait per instruction).
# ---------------------------------------------------------------------------
def _split_excess_waits(nc, max_waits=1):
    n_split = 0
    for f in nc.m.functions:
        for blk in f.blocks:
            new_insts = []
            for inst in blk.instructions:
                si = inst.sync_info
                if si is None or si.on_wait is None or len(si.on_wait) <= max_waits:
                    new_insts.append(inst)
                    continue
                waits = list(si.on_wait)
                movable = [w for w in waits if w.wait_mode == "sem-ge-imm"]
                fixed = [w for w in waits if w.wait_mode != "sem-ge-imm"]
                keep = max_waits - len(fixed)
                assert keep >= 0, f"{inst.name}: too many non-ge waits"
                kept = fixed + (movable[:keep] if keep > 0 else [])
                rest = movable[keep:] if keep > 0 else movable
                for i in range(0, len(rest), max_waits):
                    chunk = rest[i:i + max_waits]
                    n_split += 1
                    new_insts.append(mybir.InstNoOp(
                        name=f"I-waitsplit-{n_split}",
                        engine=inst.engine,
                        ins=[], outs=[],
                        sync_info=mybir.SyncInfo(on_wait=list(chunk), on_update=[]),
                        bass_nofuse=True,
                    ))
                inst.sync_info = mybir.SyncInfo(
                    on_wait=kept, on_update=list(si.on_update or []))
                new_insts.append(inst)
            blk.instructions = new_insts
    return n_split


# ---------------------------------------------------------------------------
# Device program
# ---------------------------------------------------------------------------
def _build_program():
    from contextlib import ExitStack

    nc = bass.Bass(target_bir_lowering=False, debug=False)
    xT_ext = nc.declare_dram_parameter("xT", [D, T], BF, isOutput=False)
    w_ext = nc.declare_dram_parameter("w", [D, 1536], BF, isOutput=False)
    bqk_ext = nc.declare_dram_parameter("b_qk", [128, 8], F32, isOutput=False)
    bv_ext = nc.declare_dram_parameter("b_v", [1, 512], BF, isOutput=False)
    out_ext = nc.declare_dram_parameter("out", [T, NH_CORE * HD], F32, isOutput=True)

    with tile.TileContext(nc) as tc, ExitStack() as ctx:
        const = ctx.enter_context(tc.tile_pool(name="const", bufs=1))
        # PSUM: "mm" slots are 2 banks ([128,1024] f32); fl/acc slots 1 bank
        psum_mm = ctx.enter_context(tc.tile_pool(name="psum_mm", bufs=2, space="PSUM"))
        psum_fl = ctx.enter_context(tc.tile_pool(name="psum_fl", bufs=2, space="PSUM"))
        psum_acc = ctx.enter_context(tc.tile_pool(name="psum_acc", bufs=2, space="PSUM"))
        p_pool = ctx.enter_context(tc.tile_pool(name="p_pool", bufs=6))
        ot_pool = ctx.enter_context(tc.tile_pool(name="ot_pool", bufs=3))
        o_pool = ctx.enter_context(tc.tile_pool(name="o_pool", bufs=3))
        r_pool = ctx.enter_context(tc.tile_pool(name="r_pool", bufs=2))

        # persistent SBUF tensors (split finely so Tile's per-tile dependency
        # tracking doesn't serialize phases)
        identb_sb = const.tile([128, 128], BF)
        xT_sb = [const.tile([128, T], BF, tag=f"xT{kc}", name=f"xT{kc}") for kc in range(NKC)]
        w_sb = [const.tile([128, 1536], BF, tag=f"w{kc}", name=f"w{kc}") for kc in range(NKC)]
        qt_sb = [const.tile([128, T], BF, tag=f"qt{gp}", name=f"qt{gp}") for gp in range(4)]
        kt_sb = [const.tile([128, T], BF, tag=f"kt{gp}", name=f"kt{gp}") for gp in range(4)]
        v_sb = [const.tile([128, NH_CORE * 65], BF, tag=f"v{jt}", name=f"v{jt}") for jt in range(NJT)]
        bqk_sb = const.tile([128, 8], F32)
        bv_sb = const.tile([1, 512], BF)
        ones_sb = const.tile([1, 128], BF)
        mask_sb = const.tile([128, 128], BF)
        mask2_sb = const.tile([128, 256], BF)

        # --- setup ---
        for kc in range(NKC):
            nc.gpsimd.dma_start(xT_sb[kc][:, :], xT_ext[kc * 128:(kc + 1) * 128, :])
            nc.gpsimd.dma_start(w_sb[kc][:, :], w_ext[kc * 128:(kc + 1) * 128, :])
        nc.gpsimd.dma_start(bqk_sb[:, :], bqk_ext[:, :])
        nc.gpsimd.dma_start(bv_sb[:, :], bv_ext[:, :])
        nc.vector.memset(ones_sb[:, :], 1.0)
        make_identity(nc, identb_sb[:, :])
        # causal 0/1 triangle (diagonal 128-col block): keep 1.0 where p <= f
        nc.gpsimd.memset(mask_sb[:, :], 1.0)
        nc.gpsimd.affine_select(
            out=mask_sb[:, :], in_=mask_sb[:, :],
            compare_op=mybir.AluOpType.is_ge, fill=0.0,
            base=0, pattern=[[1, 128]], channel_multiplier=-1,
        )
        nc.vector.tensor_copy(mask2_sb[:, 0:128], mask_sb[:, :])
        nc.vector.tensor_copy(mask2_sb[:, 128:256], mask_sb[:, :])
        for jt in range(NJT):
            nc.vector.memset(
                v_sb[jt][:, :].rearrange("p (h c) -> p h c", c=65)[:, :, 64:65], 1.0)

        def qk_tile_job(gp, qk, n):
            t_idx = 2 * gp + qk
            dest = qt_sb[gp] if qk == 0 else kt_sb[gp]
            ps = psum_fl.tile([128, 512], F32, tag="fl", name=f"flq{gp}_{qk}_{n}")
            for kc in range(NKC):
                nc.tensor.matmul(
                    ps[:, :],
                    lhsT=w_sb[kc][:, t_idx * 128:(t_idx + 1) * 128],
                    rhs=xT_sb[kc][:, n * 512:(n + 1) * 512],
                    start=(kc == 0), stop=(kc == NKC - 1),
                )
                yield
            nc.vector.tensor_scalar_add(
                dest[:, n * 512:(n + 1) * 512], ps[:, :],
                bqk_sb[:, t_idx:t_idx + 1],
            )
            yield

        def v_tile_job(st):
            ps = psum_fl.tile([128, 512], F32, tag="fl", name=f"flv{st}")
            nc.tensor.matmul(ps[:, :], lhsT=ones_sb[:, :], rhs=bv_sb[:, :],
                             start=True, stop=False)
            for kc in range(NKC):
                nc.tensor.matmul(
                    ps[:, :],
                    lhsT=xT_sb[kc][:, st * 128:(st + 1) * 128],
                    rhs=w_sb[kc][:, 1024:1536],
                    start=False, stop=(kc == NKC - 1),
                )
                yield
            nc.vector.tensor_copy(
                v_sb[st][:, :].rearrange("p (h c) -> p h c", c=65)[:, :, 0:64],
                ps[:, :].rearrange("p (h c) -> p h c", c=64),
            )
            yield

        est = {"pe": 0.0, "act": 0.0}

        def run_job(gen):
            for _ in gen:
                est["pe"] += 215.0

        # filler queue: [(key, generator)] pumped into phase-2 PE bubbles
        fillers = []

        def pump_one():
            while fillers:
                try:
                    next(fillers[0][1])
                    est["pe"] += 215.0
                    return True
                except StopIteration:
                    fillers.pop(0)
            return False

        def pump_balance():
            while fillers and est["pe"] < est["act"] + 2000.0:
                if not pump_one():
                    return

        def drain_through(pred):
            """Run filler jobs (FIFO) until every job matching pred is gone."""
            while any(pred(key) for key, _ in fillers):
                run_job(fillers[0][1])
                fillers.pop(0)

        pending_tail = []

        def emit_tail():
            if not pending_tail:
                return
            h, ic, ot_s = pending_tail.pop(0)
            tr_ps = psum_fl.tile([128, 264], BF, tag="fl")
            for blk in range(4):
                nc.tensor.transpose(
                    tr_ps[:, blk * 66:blk * 66 + 65],
                    ot_s[0:65, blk * 128:(blk + 1) * 128],
                    identb_sb[0:65, 0:65],
                )
            rc = r_pool.tile([128, 4], F32)
            nc.vector.reciprocal(
                rc[:, :],
                tr_ps[:, 0:264].rearrange("p (b c) -> p b c", c=66)[:, :, 64:65],
            )
            o_s = o_pool.tile([128, 256], F32)
            for blk in range(4):
                nc.vector.tensor_scalar_mul(
                    o_s[:, blk * 64:(blk + 1) * 64],
                    tr_ps[:, blk * 66:blk * 66 + 64],
                    rc[:, blk:blk + 1],
                )
            nc.gpsimd.dma_start(
                out_ext[ic * 512:(ic + 1) * 512, h * 64:(h + 1) * 64]
                .rearrange("(blk p) c -> p blk c", p=128),
                o_s[:, :].rearrange("p (blk c) -> p blk c", c=64),
            )

        def emit_unit(gp, ic):
            # both heads of the pair processed per round; their K=64 S^T
            # matmuls land on row-tiles T0/T8 (base partitions 0/64) and run
            # concurrently in the 64x128 tiled array mode
            h0 = 2 * gp
            h1 = 2 * gp + 1
            njt = 4 * ic + 4
            acc0 = psum_acc.tile([65, 512], F32, tag="acc")
            acc1 = psum_acc.tile([65, 512], F32, tag="acc")
            for jt in range(njt):
                r = jt - 4 * ic
                f0 = 128 * r if r >= 0 else 0
                st2 = psum_mm.tile([128, 1024], F32, tag="mm")
                nc.tensor.matmul(
                    st2[:, f0:512],
                    lhsT=kt_sb[gp][0:64, jt * 128:(jt + 1) * 128],
                    rhs=qt_sb[gp][0:64, ic * 512 + f0: ic * 512 + 512],
                    start=True, stop=True,
                )
                nc.tensor.matmul(
                    st2[:, 512 + f0:1024],
                    lhsT=kt_sb[gp][64:128, jt * 128:(jt + 1) * 128],
                    rhs=qt_sb[gp][64:128, ic * 512 + f0: ic * 512 + 512],
                    start=True, stop=True,
                )
                pump_one()
                p_t = p_pool.tile([128, 1024], BF, tag="pt")
                if r >= 0:
                    # diagonal tile: exp/mask only the written regions of both
                    # head-halves in single strided instructions
                    st2v = st2[:, :].rearrange("p (b c) -> p b c", c=512)[:, :, f0:512]
                    p_tv = p_t[:, :].rearrange("p (b c) -> p b c", c=512)[:, :, f0:512]
                    nc.scalar.activation(
                        p_tv, st2v, mybir.ActivationFunctionType.Exp)
                    est["act"] += (172 + 2 * (512 - f0)) / 1.2
                    p_tm = p_t[:, :].rearrange(
                        "p (b c) -> p b c", c=512)[:, :, f0:f0 + 128]
                    nc.vector.tensor_mul(
                        p_tm, p_tm,
                        mask2_sb[:, :].rearrange("p (b c) -> p b c", c=128))
                else:
                    nc.scalar.activation(
                        p_t[:, :], st2[:, :], mybir.ActivationFunctionType.Exp)
                    est["act"] += (172 + 1024) / 1.2
                nc.tensor.matmul(
                    acc0[0:65, f0:512],
                    lhsT=v_sb[jt][:, h0 * 65:(h0 + 1) * 65],
                    rhs=p_t[:, f0:512],
                    start=(jt == 0), stop=(jt == njt - 1),
                )
                nc.tensor.matmul(
                    acc1[0:65, f0:512],
                    lhsT=v_sb[jt][:, h1 * 65:(h1 + 1) * 65],
                    rhs=p_t[:, 512 + f0:1024],
                    start=(jt == 0), stop=(jt == njt - 1),
                )
                pump_one()
            for h, acc in ((h0, acc0), (h1, acc1)):
                ot_s = const.tile([65, 512], BF, tag=f"ot{h}_{ic}", name=f"ot{h}_{ic}")
                nc.vector.tensor_copy(ot_s[:, :], acc[:, :])
                pending_tail.append((h, ic, ot_s))

        # --- emission: qk(pair0) + V(0-3) upfront; the rest becomes filler
        # work pumped into phase-2 PE bubbles (keeps TensorE dense -> HAM warm)
        for qk in range(2):
            for n in range(NIC):
                run_job(qk_tile_job(0, qk, n))
        for st in range(4):
            run_job(v_tile_job(st))
        for st in range(4, 8):
            fillers.append((("v", st), v_tile_job(st)))
        for qk in range(2):
            for n in range(NIC):
                fillers.append((("qk", 1), qk_tile_job(1, qk, n)))
        for st in range(8, 12):
            fillers.append((("v", st), v_tile_job(st)))
        for qk in range(2):
            for n in range(NIC):
                fillers.append((("qk", 2), qk_tile_job(2, qk, n)))
        for st in range(12, 16):
            fillers.append((("v", st), v_tile_job(st)))
        for qk in range(2):
            for n in range(NIC):
                fillers.append((("qk", 3), qk_tile_job(3, qk, n)))

        for gp in range(4):
            for ic in range(NIC):
                drain_through(lambda key: key == ("qk", gp))
                drain_through(
                    lambda key: key[0] == "v" and key[1] <= 4 * ic + 3)
                emit_unit(gp, ic)
                while len(pending_tail) > 2:
                    emit_tail()
        while fillers:
            run_job(fillers.pop(0)[1])
        while pending_tail:
            emit_tail()

    _split_excess_waits(nc)
    return nc


_NC_CACHE = None


def _get_nc():
    global _NC_CACHE
    if _NC_CACHE is None:
        _NC_CACHE = _build_program()
    return _NC_CACHE


# ---------------------------------------------------------------------------
# Host-side sharding / unsharding
# ---------------------------------------------------------------------------
def _make_in_maps(x, W_qkv, b_qkv):
    scale = 1.0 / np.sqrt(np.float32(D))
    Wq, Wk, Wv = W_qkv[:, 0:D], W_qkv[:, D:2 * D], W_qkv[:, 2 * D:3 * D]
    bq, bk, bv = b_qkv[0:D], b_qkv[D:2 * D], b_qkv[2 * D:3 * D]
    in_maps = []
    for c in range(NCORES):
        b, g2 = divmod(c, 2)
        h0 = NH_CORE * g2  # first global head of this core
        xT = np.ascontiguousarray(x[b].T).astype(BF16)
        w_cols = []
        bqk_cols = []
        for gp in range(4):
            lo = (h0 + 2 * gp) * HD
            hi = lo + 2 * HD
            w_cols.append((Wq[:, lo:hi] * scale))
            w_cols.append(Wk[:, lo:hi])
            bqk_cols.append(bq[lo:hi] * scale)
            bqk_cols.append(bk[lo:hi])
        w_cols.append(Wv[:, h0 * HD:(h0 + NH_CORE) * HD])
        w = np.concatenate(w_cols, axis=1).astype(BF16)       # [1024, 1536]
        b_qk = np.stack(bqk_cols, axis=1).astype(np.float32)   # [128, 8]
        b_v = bv[h0 * HD:(h0 + NH_CORE) * HD].reshape(1, 512).astype(BF16)
        in_maps.append({"xT": xT, "w": w, "b_qk": b_qk, "b_v": b_v})
    return in_maps


def run(x, W_qkv, b_qkv, trace=False):
    """Run the distributed kernel; returns (out, BassKernelResults)."""
    nc = _get_nc()
    in_maps = _make_in_maps(np.asarray(x), np.asarray(W_qkv), np.asarray(b_qkv))
    res = run_bass_kernel_spmd(nc, in_maps, core_ids=list(range(NCORES)),
                               trace=trace)
    out = np.empty((4, T, D), dtype=np.float32)
    for c in range(NCORES):
        b, g2 = divmod(c, 2)
        out[b, :, g2 * 512:(g2 + 1) * 512] = res.results[c]["out"]
    return out, res


def kernel(x, W_qkv, b_qkv):
    out, _ = run(x, W_qkv, b_qkv, trace=False)
    return out
